# revision 13
# baseline (speedup 1.0000x reference)
"""Trainium2 Bass kernel for nn_PlanningLoss_21397527069385.

Strategy (pure data parallelism over batch, 8 cores x 1024 images):
  - Host packs both BEV masks into one x-transposed, x-padded, bf16 tensor
    combo[b, 4+x, 0:48] = det[b, :, x], combo[b, 4+x, 64:112] = driv[b, :, x].
    0/1 mask values are exact in bf16.
  - Per trajectory point (b, s), one 2304B dma_gather descriptor fetches the 9
    x-columns around px, which contains BOTH the 11x9 collision window and the
    7x5 lane window (all y values present in each column).
  - DVE extracts windowed sums with per-partition dynamic y-masks
    (scalar_tensor_tensor with accum_out); collision risk = min(sum, 1)
    since the mask is 0/1 (window max == clamped window sum).
  - Trajectory MSE / smoothness / confidence terms computed from small tiles.
  - Per-core partial sums reduced over partitions with one matmul; host
    combines the 8 cores' partials into the final scalar (the all-reduce).
"""

import numpy as np
import ml_dtypes

import concourse.bass as bass
import concourse.bacc as bacc
import concourse.mybir as mybir
from concourse import tile
from concourse.bass_utils import run_bass_kernel_spmd
from concourse.library_config import mlp as mlp_lib

F32 = mybir.dt.float32
BF16 = mybir.dt.bfloat16
I16 = mybir.dt.int16
I32 = mybir.dt.int32
OP = mybir.AluOpType

# Problem constants
B, S, H, W = 8192, 6, 48, 80
NC = 8
BC = B // NC                # 1024 images per core
PTS = BC * S                # 6144 points per core
CHUNKS = PTS // 128         # 48 point-chunks of 128
XP, YP, ROWE = 88, 64, 128  # padded x-rows, padded y, row elems (det 64 + driv 64)
SLICE = 256                 # images per gather source slice (int16 index range)
NSLICE = BC // SLICE        # 4
PER_CALL = 768              # gather descriptors per dma_gather call
CALLS_PER_SLICE = (SLICE * S) // PER_CALL   # 2
NCALLS = NSLICE * CALLS_PER_SLICE           # 8
BLK = 9 * ROWE              # 1152 bf16 elems = 2304B per gathered block

W_TRAJ, W_COL, W_LANE, W_SMOOTH, W_CONF = 1.0, 5.0, 2.0, 0.5, 0.1
AUXW = 400

TRACE = False
LAST_EXEC_NS = None

_NC_CACHE = None

# point i = b_local*6 + s.  Extraction layout: point i -> [i%128, i//128].
# Gather idx wrap layout: for call c, wrap col s16 (0..31), partition p16:
# point i = c*512 + s16*16 + p16.  Columns f = c*32 + s16 of a [128, 384] tile.
_PP = np.arange(128)
_CC = np.arange(CHUNKS)
_POINT_P = (_CC[None, :] * 128 + _PP[:, None])          # [128, 48] point ids
_WC = PER_CALL // 16
_FF = np.arange(NCALLS * _WC)
_IOY = np.tile(np.arange(-2, YP - 2, dtype=np.float32), (128, 1))
_POINT_W = ((_FF[None, :] // _WC) * PER_CALL + (_FF[None, :] % _WC) * 16
            + (_PP[:, None] % 16))                      # [128, 384] point ids


def _floor_clamp(nc, pool, x, lo, hi, ncols):
    """floor(x) then clamp to [lo, hi]; robust to any HW cast rounding."""
    i32 = pool.tile([128, ncols], I32, tag="fc_i32")
    nc.vector.tensor_copy(i32[:], x[:])
    i_f = pool.tile([128, ncols], F32, tag="fc_if")
    nc.vector.tensor_copy(i_f[:], i32[:])
    gt = pool.tile([128, ncols], F32, tag="fc_gt")
    nc.vector.tensor_tensor(out=gt[:], in0=i_f[:], in1=x[:], op=OP.is_gt)
    fl = pool.tile([128, ncols], F32, tag="fc_fl")
    nc.vector.tensor_tensor(out=fl[:], in0=i_f[:], in1=gt[:], op=OP.subtract)
    out = pool.tile([128, ncols], F32, tag="fc_out")
    nc.vector.tensor_scalar(out=out[:], in0=fl[:], scalar1=float(lo),
                            scalar2=float(hi), op0=OP.max, op1=OP.min)
    return out


def _build_nc():
    nc = bacc.Bacc("TRN2", target_bir_lowering=False, debug=False, num_devices=NC)
    combo = nc.dram_tensor("combo", [BC * XP, ROWE], BF16, kind="ExternalInput").ap()
    auxD = nc.dram_tensor("auxD", [128, AUXW], F32, kind="ExternalInput").ap()
    idxD = nc.dram_tensor("idxD", [128, NCALLS * _WC], I16, kind="ExternalInput").ap()
    partials = nc.dram_tensor("partials", [8, 1], F32, kind="ExternalOutput").ap()

    combo_flat = combo.rearrange("a b -> (a b)")

    with tile.TileContext(nc) as tc:
        with (
            tc.tile_pool(name="aux", bufs=1) as aux,
            tc.tile_pool(name="gat", bufs=4) as gat,
            tc.tile_pool(name="work", bufs=1) as work,
            tc.tile_pool(name="psum", bufs=1, space="PSUM") as psum,
        ):
            # Load the gather ucode library first so the reload (~5us)
            # overlaps the input DMAs instead of gating the first gather.
            nc.gpsimd.load_library(mlp_lib)

            # Tiny dummy gather absorbs one-time SWDGE/ucode init cost
            # while the real input DMAs are still in flight.
            dummy_idx = aux.tile([128, 8], I16)
            nc.vector.memset(dummy_idx[:], 0)
            dummy_dst = aux.tile([128, 1, BLK], BF16)
            dummy_view = bass.AP(combo_flat.tensor, combo_flat.offset,
                                 [[ROWE, 16], [1, BLK]])
            nc.gpsimd.dma_gather(dummy_dst[:], dummy_view, dummy_idx[:],
                                 128, 128, BLK, elem_step=ROWE)

            # ---- load packed aux + indices (two DMAs) ----
            idx16 = aux.tile([128, NCALLS * _WC], I16)
            nc.sync.dma_start(idx16[:], idxD[:])
            auxT = aux.tile([128, AUXW], F32)
            nc.sync.dma_start(auxT[:], auxD[:])
            tx = auxT[:, 0:CHUNKS]
            ty = auxT[:, CHUNKS:2 * CHUNKS]
            cf = auxT[:, 2 * CHUNKS:3 * CHUNKS]
            pred = auxT[:, 144:240]
            targ = auxT[:, 240:336]
            ioy = auxT[:, 336:336 + YP]

            # ---- issue all gathers first (Pool desc-gen is the critical path) ----
            spc = SLICE * XP * ROWE  # elems per slice of combo
            dsts = []
            for c in range(NCALLS):
                sl = c // CALLS_PER_SLICE
                src_view = bass.AP(combo_flat.tensor, combo_flat.offset + sl * spc,
                                   [[ROWE, SLICE * XP - 8], [1, BLK]])
                dst = gat.tile([128, PER_CALL // 128, BLK], BF16, tag="dst")
                nc.gpsimd.dma_gather(
                    dst[:], src_view, idx16[:, c * _WC:(c + 1) * _WC],
                    PER_CALL, PER_CALL, BLK, elem_step=ROWE)
                dsts.append(dst)

            # ---- px/py and window bounds in point layout [128, 48] ----
            fx = work.tile([128, CHUNKS], F32, tag="fx")
            nc.vector.tensor_scalar(out=fx[:], in0=tx, scalar1=2.0,
                                    scalar2=40.0, op0=OP.mult, op1=OP.add)
            px = _floor_clamp(nc, work, fx, 0, W - 1, CHUNKS)
            fy = work.tile([128, CHUNKS], F32, tag="fy")
            nc.vector.tensor_scalar(out=fy[:], in0=ty, scalar1=1.6,
                                    scalar2=24.0, op0=OP.mult, op1=OP.add)
            py = _floor_clamp(nc, work, fy, 0, H - 1, CHUNKS)

            def bound(src, ofs, lo, hi, tag):
                t = work.tile([128, CHUNKS], F32, tag=tag)
                nc.vector.tensor_scalar(out=t[:], in0=src[:], scalar1=float(ofs),
                                        scalar2=float(lo), op0=OP.add, op1=OP.max)
                t2 = work.tile([128, CHUNKS], F32, tag=tag + "b")
                nc.vector.tensor_scalar(out=t2[:], in0=t[:], scalar1=float(hi),
                                        scalar2=None, op0=OP.min)
                return t2

            yloU = work.tile([128, CHUNKS], F32, tag="yloU")
            nc.vector.tensor_scalar(out=yloU[:], in0=py[:], scalar1=-5.0,
                                    scalar2=None, op0=OP.add)
            yhiU = work.tile([128, CHUNKS], F32, tag="yhiU")
            nc.vector.tensor_scalar(out=yhiU[:], in0=py[:], scalar1=6.0,
                                    scalar2=None, op0=OP.add)
            yloL = bound(py, -3, 0, H, "yloL")
            yhiL = bound(py, 4, 0, H, "yhiL")
            xloL = bound(px, -2, 0, W, "xloL")
            xhiL = bound(px, 3, 0, W, "xhiL")

            cntx = work.tile([128, CHUNKS], F32, tag="cntx")
            nc.vector.tensor_tensor(out=cntx[:], in0=xhiL[:], in1=xloL[:], op=OP.subtract)
            cnty = work.tile([128, CHUNKS], F32, tag="cnty")
            nc.vector.tensor_tensor(out=cnty[:], in0=yhiL[:], in1=yloL[:], op=OP.subtract)
            cnt = work.tile([128, CHUNKS], F32, tag="cnt")
            nc.vector.tensor_tensor(out=cnt[:], in0=cntx[:], in1=cnty[:], op=OP.mult)
            icnt = work.tile([128, CHUNKS], F32, tag="icnt")
            nc.vector.reciprocal(icnt[:], cnt[:])

            # ---- per-point y-window masks for all chunks: [128, 48, 64] ----

            # wide collision mask over y in [-2, 50): ymc52[.., j] = [py-5 <= j-2 < py+6]
            HW2 = H + 4
            iob = ioy[:, None, 0:HW2].broadcast_to((128, CHUNKS, HW2))
            m1 = work.tile([128, CHUNKS, HW2], BF16, tag="ym_m1")
            nc.vector.tensor_tensor(
                out=m1[:], in0=iob,
                in1=yloU[:, :, None].broadcast_to((128, CHUNKS, HW2)), op=OP.is_ge)
            m2 = work.tile([128, CHUNKS, HW2], BF16, tag="ym_m2")
            nc.vector.tensor_tensor(
                out=m2[:], in0=iob,
                in1=yhiU[:, :, None].broadcast_to((128, CHUNKS, HW2)), op=OP.is_lt)
            ymc52 = work.tile([128, CHUNKS, HW2], BF16, tag="ymc52")
            nc.vector.tensor_tensor(out=ymc52[:], in0=m1[:], in1=m2[:], op=OP.mult)
            ymc = ymc52[:, :, 2:2 + H]
            # lane mask = ymc(y+2) * ymc(y-2) = [py-3 <= y < py+4]; y-clamps are
            # no-ops over y in [0,48) so the unclamped identity is exact.
            yml = work.tile([128, CHUNKS, H], BF16, tag="yml")
            nc.vector.tensor_tensor(out=yml[:], in0=ymc52[:, :, 4:4 + H],
                                    in1=ymc52[:, :, 0:H], op=OP.mult)

            # ---- extraction ----
            colacc = work.tile([128, CHUNKS], F32, tag="colacc")
            laneacc = work.tile([128, CHUNKS], F32, tag="laneacc")
            for c in range(NCALLS):
                dst = dsts[c]
                for k in range(PER_CALL // 128):
                    g = c * (PER_CALL // 128) + k
                    blk = dst[:, k, :].rearrange("p (a b) -> p a b", a=9)
                    s9 = gat.tile([128, 9, H], BF16, tag="s9")
                    nc.vector.scalar_tensor_tensor(
                        out=s9[:], in0=blk[:, :, 0:H], scalar=1.0,
                        in1=ymc[:, g:g + 1, :].broadcast_to((128, 9, H)),
                        op0=OP.mult, op1=OP.mult,
                        accum_out=colacc[:, g:g + 1])
                    s5 = gat.tile([128, 5, H], BF16, tag="s5")
                    nc.vector.scalar_tensor_tensor(
                        out=s5[:], in0=blk[:, 2:7, YP:YP + H], scalar=1.0,
                        in1=yml[:, g:g + 1, :].broadcast_to((128, 5, H)),
                        op0=OP.mult, op1=OP.mult,
                        accum_out=laneacc[:, g:g + 1])

            # ---- per-point finals -> per-partition partial sums ----
            parts = aux.tile([128, 8], F32)
            nc.vector.memset(parts[:], 0.0)

            risk = work.tile([128, CHUNKS], F32, tag="risk")
            nc.vector.tensor_scalar(out=risk[:], in0=colacc[:], scalar1=1.0,
                                    scalar2=None, op0=OP.min)
            nc.vector.tensor_reduce(out=parts[:, 2:3], in_=risk[:],
                                    axis=mybir.AxisListType.X, op=OP.add)
            lanec = work.tile([128, CHUNKS], F32, tag="lanec")
            nc.vector.tensor_tensor(out=lanec[:], in0=laneacc[:], in1=icnt[:], op=OP.mult)
            nc.vector.tensor_reduce(out=parts[:, 3:4], in_=lanec[:],
                                    axis=mybir.AxisListType.X, op=OP.add)
            cerr = work.tile([128, CHUNKS], F32, tag="cerr")
            nc.vector.tensor_scalar(out=cerr[:], in0=cf, scalar1=-1.0,
                                    scalar2=None, op0=OP.add)
            nc.vector.tensor_tensor(out=cerr[:], in0=cerr[:], in1=risk[:], op=OP.add)
            csq = work.tile([128, CHUNKS], F32, tag="csq")
            nc.scalar.activation(csq[:], cerr[:], mybir.ActivationFunctionType.Square,
                                 accum_out=parts[:, 4:5])

            # trajectory MSE partial
            td = work.tile([128, 96], F32, tag="td")
            nc.vector.tensor_tensor(out=td[:], in0=pred, in1=targ, op=OP.subtract)
            tsq = work.tile([128, 96], F32, tag="tsq")
            nc.scalar.activation(tsq[:], td[:], mybir.ActivationFunctionType.Square,
                                 accum_out=parts[:, 0:1])

            # smoothness partial
            pv = pred.rearrange("p (n d) -> p n d", n=8)
            xs_ = pv[:, :, 0:12:2]
            ys_ = pv[:, :, 1:12:2]
            t1 = work.tile([128, 8, 4], F32, tag="t1")
            t2 = work.tile([128, 8, 4], F32, tag="t2")
            ax = work.tile([128, 8, 4], F32, tag="ax")
            ay = work.tile([128, 8, 4], F32, tag="ay")
            nc.vector.tensor_tensor(out=t1[:], in0=xs_[:, :, 2:6], in1=xs_[:, :, 1:5], op=OP.subtract)
            nc.vector.tensor_tensor(out=t2[:], in0=xs_[:, :, 1:5], in1=xs_[:, :, 0:4], op=OP.subtract)
            nc.vector.tensor_tensor(out=ax[:], in0=t1[:], in1=t2[:], op=OP.subtract)
            nc.vector.tensor_tensor(out=t1[:], in0=ys_[:, :, 2:6], in1=ys_[:, :, 1:5], op=OP.subtract)
            nc.vector.tensor_tensor(out=t2[:], in0=ys_[:, :, 1:5], in1=ys_[:, :, 0:4], op=OP.subtract)
            nc.vector.tensor_tensor(out=ay[:], in0=t1[:], in1=t2[:], op=OP.subtract)
            nc.vector.tensor_tensor(out=ax[:], in0=ax[:], in1=ax[:], op=OP.mult)
            nc.vector.tensor_tensor(out=ay[:], in0=ay[:], in1=ay[:], op=OP.mult)
            nc.vector.tensor_tensor(out=ax[:], in0=ax[:], in1=ay[:], op=OP.add)
            sm = work.tile([128, 8, 4], F32, tag="sm")
            nc.scalar.activation(sm[:], ax[:], mybir.ActivationFunctionType.Sqrt,
                                 accum_out=parts[:, 1:2])

            # ---- partition reduce via matmul, then out ----
            ones = aux.tile([128, 1], F32)
            nc.vector.memset(ones[:], 1.0)
            red = psum.tile([8, 1], F32)
            nc.tensor.matmul(red[:], parts[:], ones[:], start=True, stop=True)
            sred = aux.tile([8, 1], F32)
            nc.scalar.copy(out=sred[:], in_=red[:])
            nc.sync.dma_start(partials[:], sred[:])

    nc.compile()
    return nc


def _host_prep(predicted_trajectory, predicted_confidence, target_trajectory,
               detection_mask, drivable_area_mask):
    in_maps = []
    for k in range(NC):
        b0, b1 = k * BC, (k + 1) * BC
        det = detection_mask[b0:b1]
        drv = drivable_area_mask[b0:b1]
        combo = np.zeros((BC, XP, ROWE), dtype=ml_dtypes.bfloat16)
        combo[:, 4:4 + W, 0:H] = det.transpose(0, 2, 1)
        combo[:, 4:4 + W, YP:YP + H] = drv.transpose(0, 2, 1)

        ptx = np.ascontiguousarray(predicted_trajectory[b0:b1, :, 0]).reshape(-1)
        pty = np.ascontiguousarray(predicted_trajectory[b0:b1, :, 1]).reshape(-1)
        cnf = np.ascontiguousarray(predicted_confidence[b0:b1]).reshape(-1)

        pxw = np.clip((ptx[_POINT_W] / 0.5 + 40.0).astype(np.int32), 0, W - 1)
        base = ((_POINT_W // S) % SLICE) * XP
        idx16 = (base + pxw).astype(np.int16)
        pr = predicted_trajectory[b0:b1].reshape(8, 128, 12).transpose(1, 0, 2)
        tg = target_trajectory[b0:b1].reshape(8, 128, 12).transpose(1, 0, 2)
        auxA = np.empty((128, AUXW), np.float32)
        auxA[:, 0:CHUNKS] = ptx[_POINT_P]
        auxA[:, CHUNKS:2 * CHUNKS] = pty[_POINT_P]
        auxA[:, 2 * CHUNKS:3 * CHUNKS] = cnf[_POINT_P]
        auxA[:, 144:240] = pr.reshape(128, 96)
        auxA[:, 240:336] = tg.reshape(128, 96)
        auxA[:, 336:336 + YP] = _IOY
        auxA[:, 336 + YP:] = 0.0

        in_maps.append({
            "combo": combo.reshape(BC * XP, ROWE),
            "auxD": auxA,
            "idxD": idx16,
        })
    return in_maps


def kernel(predicted_trajectory, predicted_confidence, target_trajectory,
           detection_mask, drivable_area_mask):
    global _NC_CACHE, LAST_EXEC_NS
    if _NC_CACHE is None:
        _NC_CACHE = _build_nc()
    nc = _NC_CACHE
    in_maps = _host_prep(predicted_trajectory, predicted_confidence,
                         target_trajectory, detection_mask, drivable_area_mask)
    res = run_bass_kernel_spmd(nc, in_maps, list(range(NC)), trace=TRACE)
    LAST_EXEC_NS = res.exec_time_ns
    tot = np.zeros(8, dtype=np.float64)
    for r in res.results:
        tot += r["partials"].reshape(-1).astype(np.float64)
    traj_loss = tot[0] / (B * S * 2)
    smooth_loss = tot[1] / (B * (S - 2))
    col_loss = tot[2] / (B * S)
    lane_loss = 1.0 - tot[3] / (B * S)
    conf_loss = tot[4] / (B * S)
    total = (W_TRAJ * traj_loss + W_COL * col_loss + W_LANE * lane_loss
             + W_SMOOTH * smooth_loss + W_CONF * conf_loss)
    return np.float32(total)


# revision 14
# speedup vs baseline: 1.0413x; 1.0413x over previous
"""Trainium2 Bass kernel for nn_PlanningLoss_21397527069385.

Strategy (pure data parallelism over batch, 8 cores x 1024 images):
  - Host packs both BEV masks into one x-transposed, x-padded, bf16 tensor
    combo[b, 4+x, 0:48] = det[b, :, x], combo[b, 4+x, 64:112] = driv[b, :, x].
    0/1 mask values are exact in bf16.
  - Per trajectory point (b, s), one 2304B dma_gather descriptor fetches the 9
    x-columns around px, which contains BOTH the 11x9 collision window and the
    7x5 lane window (all y values present in each column).
  - DVE extracts windowed sums with per-partition dynamic y-masks
    (scalar_tensor_tensor with accum_out); collision risk = min(sum, 1)
    since the mask is 0/1 (window max == clamped window sum).
  - Trajectory MSE / smoothness / confidence terms computed from small tiles.
  - Per-core partial sums reduced over partitions with one matmul; host
    combines the 8 cores' partials into the final scalar (the all-reduce).
"""

import numpy as np
import ml_dtypes

import concourse.bass as bass
import concourse.bacc as bacc
import concourse.mybir as mybir
from concourse import tile
from concourse.bass_utils import run_bass_kernel_spmd
from concourse.library_config import mlp as mlp_lib

F32 = mybir.dt.float32
BF16 = mybir.dt.bfloat16
I16 = mybir.dt.int16
I32 = mybir.dt.int32
OP = mybir.AluOpType

# Problem constants
B, S, H, W = 8192, 6, 48, 80
NC = 8
BC = B // NC                # 1024 images per core
PTS = BC * S                # 6144 points per core
CHUNKS = PTS // 128         # 48 point-chunks of 128
XP, YP, ROWE = 88, 64, 128  # padded x-rows, padded y, row elems (det 64 + driv 64)
SLICE = 256                 # images per gather source slice (int16 index range)
NSLICE = BC // SLICE        # 4
PER_CALL = 768              # gather descriptors per dma_gather call
CALLS_PER_SLICE = (SLICE * S) // PER_CALL   # 2
NCALLS = NSLICE * CALLS_PER_SLICE           # 8
BLK = 9 * ROWE              # 1152 bf16 elems = 2304B per gathered block

W_TRAJ, W_COL, W_LANE, W_SMOOTH, W_CONF = 1.0, 5.0, 2.0, 0.5, 0.1
AUXW = 400

TRACE = False
LAST_EXEC_NS = None

_NC_CACHE = None

# point i = b_local*6 + s.  Extraction layout: point i -> [i%128, i//128].
# Gather idx wrap layout: for call c, wrap col s16 (0..31), partition p16:
# point i = c*512 + s16*16 + p16.  Columns f = c*32 + s16 of a [128, 384] tile.
_PP = np.arange(128)
_CC = np.arange(CHUNKS)
_POINT_P = (_CC[None, :] * 128 + _PP[:, None])          # [128, 48] point ids
_WC = PER_CALL // 16
_FF = np.arange(NCALLS * _WC)
_IOY = np.tile(np.arange(-2, YP - 2, dtype=np.float32), (128, 1))
_POINT_W = ((_FF[None, :] // _WC) * PER_CALL + (_FF[None, :] % _WC) * 16
            + (_PP[:, None] % 16))                      # [128, 384] point ids


def _floor_clamp(nc, pool, x, lo, hi, ncols):
    """floor(x) then clamp to [lo, hi]; robust to any HW cast rounding."""
    i32 = pool.tile([128, ncols], I32, tag="fc_i32")
    nc.vector.tensor_copy(i32[:], x[:])
    i_f = pool.tile([128, ncols], F32, tag="fc_if")
    nc.vector.tensor_copy(i_f[:], i32[:])
    gt = pool.tile([128, ncols], F32, tag="fc_gt")
    nc.vector.tensor_tensor(out=gt[:], in0=i_f[:], in1=x[:], op=OP.is_gt)
    fl = pool.tile([128, ncols], F32, tag="fc_fl")
    nc.vector.tensor_tensor(out=fl[:], in0=i_f[:], in1=gt[:], op=OP.subtract)
    out = pool.tile([128, ncols], F32, tag="fc_out")
    nc.vector.tensor_scalar(out=out[:], in0=fl[:], scalar1=float(lo),
                            scalar2=float(hi), op0=OP.max, op1=OP.min)
    return out


def _build_nc():
    nc = bacc.Bacc("TRN2", target_bir_lowering=False, debug=False, num_devices=NC)
    combo = nc.dram_tensor("combo", [BC * XP, ROWE], BF16, kind="ExternalInput").ap()
    auxD = nc.dram_tensor("auxD", [128, AUXW], F32, kind="ExternalInput").ap()
    idxD = nc.dram_tensor("idxD", [128, NCALLS * _WC], I16, kind="ExternalInput").ap()
    partials = nc.dram_tensor("partials", [8, 1], F32, kind="ExternalOutput").ap()

    combo_flat = combo.rearrange("a b -> (a b)")

    with tile.TileContext(nc) as tc:
        with (
            tc.tile_pool(name="aux", bufs=1) as aux,
            tc.tile_pool(name="gat", bufs=4) as gat,
            tc.tile_pool(name="work", bufs=1) as work,
            tc.tile_pool(name="psum", bufs=1, space="PSUM") as psum,
        ):
            # Load the gather ucode library first so the reload (~5us)
            # overlaps the input DMAs instead of gating the first gather.
            nc.gpsimd.load_library(mlp_lib)

            # ---- load packed aux + indices (two DMAs) ----
            idx16 = aux.tile([128, NCALLS * _WC], I16)
            nc.sync.dma_start(idx16[:], idxD[:])
            auxT = aux.tile([128, AUXW], F32)
            nc.sync.dma_start(auxT[:], auxD[:])
            tx = auxT[:, 0:CHUNKS]
            ty = auxT[:, CHUNKS:2 * CHUNKS]
            cf = auxT[:, 2 * CHUNKS:3 * CHUNKS]
            pred = auxT[:, 144:240]
            targ = auxT[:, 240:336]
            ioy = auxT[:, 336:336 + YP]

            # ---- issue all gathers first (Pool desc-gen is the critical path) ----
            spc = SLICE * XP * ROWE  # elems per slice of combo
            dsts = []
            for c in range(NCALLS):
                sl = c // CALLS_PER_SLICE
                src_view = bass.AP(combo_flat.tensor, combo_flat.offset + sl * spc,
                                   [[ROWE, SLICE * XP - 8], [1, BLK]])
                dst = gat.tile([128, PER_CALL // 128, BLK], BF16, tag="dst")
                nc.gpsimd.dma_gather(
                    dst[:], src_view, idx16[:, c * _WC:(c + 1) * _WC],
                    PER_CALL, PER_CALL, BLK, elem_step=ROWE)
                dsts.append(dst)

            # ---- px/py and window bounds in point layout [128, 48] ----
            fx = work.tile([128, CHUNKS], F32, tag="fx")
            nc.vector.tensor_scalar(out=fx[:], in0=tx, scalar1=2.0,
                                    scalar2=40.0, op0=OP.mult, op1=OP.add)
            px = _floor_clamp(nc, work, fx, 0, W - 1, CHUNKS)
            fy = work.tile([128, CHUNKS], F32, tag="fy")
            nc.vector.tensor_scalar(out=fy[:], in0=ty, scalar1=1.6,
                                    scalar2=24.0, op0=OP.mult, op1=OP.add)
            py = _floor_clamp(nc, work, fy, 0, H - 1, CHUNKS)

            def bound(src, ofs, lo, hi, tag):
                t = work.tile([128, CHUNKS], F32, tag=tag)
                nc.vector.tensor_scalar(out=t[:], in0=src[:], scalar1=float(ofs),
                                        scalar2=float(lo), op0=OP.add, op1=OP.max)
                t2 = work.tile([128, CHUNKS], F32, tag=tag + "b")
                nc.vector.tensor_scalar(out=t2[:], in0=t[:], scalar1=float(hi),
                                        scalar2=None, op0=OP.min)
                return t2

            yloU = work.tile([128, CHUNKS], F32, tag="yloU")
            nc.vector.tensor_scalar(out=yloU[:], in0=py[:], scalar1=-5.0,
                                    scalar2=None, op0=OP.add)
            yhiU = work.tile([128, CHUNKS], F32, tag="yhiU")
            nc.vector.tensor_scalar(out=yhiU[:], in0=py[:], scalar1=6.0,
                                    scalar2=None, op0=OP.add)
            yloL = bound(py, -3, 0, H, "yloL")
            yhiL = bound(py, 4, 0, H, "yhiL")
            xloL = bound(px, -2, 0, W, "xloL")
            xhiL = bound(px, 3, 0, W, "xhiL")

            cntx = work.tile([128, CHUNKS], F32, tag="cntx")
            nc.vector.tensor_tensor(out=cntx[:], in0=xhiL[:], in1=xloL[:], op=OP.subtract)
            cnty = work.tile([128, CHUNKS], F32, tag="cnty")
            nc.vector.tensor_tensor(out=cnty[:], in0=yhiL[:], in1=yloL[:], op=OP.subtract)
            cnt = work.tile([128, CHUNKS], F32, tag="cnt")
            nc.vector.tensor_tensor(out=cnt[:], in0=cntx[:], in1=cnty[:], op=OP.mult)
            icnt = work.tile([128, CHUNKS], F32, tag="icnt")
            nc.vector.reciprocal(icnt[:], cnt[:])

            # ---- per-point y-window masks for all chunks: [128, 48, 64] ----

            # wide collision mask over y in [-2, 50): ymc52[.., j] = [py-5 <= j-2 < py+6]
            HW2 = H + 4
            iob = ioy[:, None, 0:HW2].broadcast_to((128, CHUNKS, HW2))
            m1 = work.tile([128, CHUNKS, HW2], BF16, tag="ym_m1")
            nc.vector.tensor_tensor(
                out=m1[:], in0=iob,
                in1=yloU[:, :, None].broadcast_to((128, CHUNKS, HW2)), op=OP.is_ge)
            m2 = work.tile([128, CHUNKS, HW2], BF16, tag="ym_m2")
            nc.vector.tensor_tensor(
                out=m2[:], in0=iob,
                in1=yhiU[:, :, None].broadcast_to((128, CHUNKS, HW2)), op=OP.is_lt)
            ymc52 = work.tile([128, CHUNKS, HW2], BF16, tag="ymc52")
            nc.vector.tensor_tensor(out=ymc52[:], in0=m1[:], in1=m2[:], op=OP.mult)
            ymc = ymc52[:, :, 2:2 + H]
            # lane mask = ymc(y+2) * ymc(y-2) = [py-3 <= y < py+4]; y-clamps are
            # no-ops over y in [0,48) so the unclamped identity is exact.
            yml = work.tile([128, CHUNKS, H], BF16, tag="yml")
            nc.vector.tensor_tensor(out=yml[:], in0=ymc52[:, :, 4:4 + H],
                                    in1=ymc52[:, :, 0:H], op=OP.mult)

            # ---- extraction ----
            colacc = work.tile([128, CHUNKS], F32, tag="colacc")
            laneacc = work.tile([128, CHUNKS], F32, tag="laneacc")
            for c in range(NCALLS):
                dst = dsts[c]
                for k in range(PER_CALL // 128):
                    g = c * (PER_CALL // 128) + k
                    blk = dst[:, k, :].rearrange("p (a b) -> p a b", a=9)
                    s9 = gat.tile([128, 9, H], BF16, tag="s9")
                    nc.vector.scalar_tensor_tensor(
                        out=s9[:], in0=blk[:, :, 0:H], scalar=1.0,
                        in1=ymc[:, g:g + 1, :].broadcast_to((128, 9, H)),
                        op0=OP.mult, op1=OP.mult,
                        accum_out=colacc[:, g:g + 1])
                    s5 = gat.tile([128, 5, H], BF16, tag="s5")
                    nc.vector.scalar_tensor_tensor(
                        out=s5[:], in0=blk[:, 2:7, YP:YP + H], scalar=1.0,
                        in1=yml[:, g:g + 1, :].broadcast_to((128, 5, H)),
                        op0=OP.mult, op1=OP.mult,
                        accum_out=laneacc[:, g:g + 1])

            # ---- per-point finals -> per-partition partial sums ----
            parts = aux.tile([128, 8], F32)
            nc.vector.memset(parts[:], 0.0)

            risk = work.tile([128, CHUNKS], F32, tag="risk")
            nc.vector.tensor_scalar(out=risk[:], in0=colacc[:], scalar1=1.0,
                                    scalar2=None, op0=OP.min)
            nc.vector.tensor_reduce(out=parts[:, 2:3], in_=risk[:],
                                    axis=mybir.AxisListType.X, op=OP.add)
            lanec = work.tile([128, CHUNKS], F32, tag="lanec")
            nc.vector.tensor_tensor(out=lanec[:], in0=laneacc[:], in1=icnt[:], op=OP.mult)
            nc.vector.tensor_reduce(out=parts[:, 3:4], in_=lanec[:],
                                    axis=mybir.AxisListType.X, op=OP.add)
            cerr = work.tile([128, CHUNKS], F32, tag="cerr")
            nc.vector.tensor_scalar(out=cerr[:], in0=cf, scalar1=-1.0,
                                    scalar2=None, op0=OP.add)
            nc.vector.tensor_tensor(out=cerr[:], in0=cerr[:], in1=risk[:], op=OP.add)
            csq = work.tile([128, CHUNKS], F32, tag="csq")
            nc.scalar.activation(csq[:], cerr[:], mybir.ActivationFunctionType.Square,
                                 accum_out=parts[:, 4:5])

            # trajectory MSE partial
            td = work.tile([128, 96], F32, tag="td")
            nc.vector.tensor_tensor(out=td[:], in0=pred, in1=targ, op=OP.subtract)
            tsq = work.tile([128, 96], F32, tag="tsq")
            nc.scalar.activation(tsq[:], td[:], mybir.ActivationFunctionType.Square,
                                 accum_out=parts[:, 0:1])

            # smoothness partial
            pv = pred.rearrange("p (n d) -> p n d", n=8)
            xs_ = pv[:, :, 0:12:2]
            ys_ = pv[:, :, 1:12:2]
            t1 = work.tile([128, 8, 4], F32, tag="t1")
            t2 = work.tile([128, 8, 4], F32, tag="t2")
            ax = work.tile([128, 8, 4], F32, tag="ax")
            ay = work.tile([128, 8, 4], F32, tag="ay")
            nc.vector.tensor_tensor(out=t1[:], in0=xs_[:, :, 2:6], in1=xs_[:, :, 1:5], op=OP.subtract)
            nc.vector.tensor_tensor(out=t2[:], in0=xs_[:, :, 1:5], in1=xs_[:, :, 0:4], op=OP.subtract)
            nc.vector.tensor_tensor(out=ax[:], in0=t1[:], in1=t2[:], op=OP.subtract)
            nc.vector.tensor_tensor(out=t1[:], in0=ys_[:, :, 2:6], in1=ys_[:, :, 1:5], op=OP.subtract)
            nc.vector.tensor_tensor(out=t2[:], in0=ys_[:, :, 1:5], in1=ys_[:, :, 0:4], op=OP.subtract)
            nc.vector.tensor_tensor(out=ay[:], in0=t1[:], in1=t2[:], op=OP.subtract)
            nc.vector.tensor_tensor(out=ax[:], in0=ax[:], in1=ax[:], op=OP.mult)
            nc.vector.tensor_tensor(out=ay[:], in0=ay[:], in1=ay[:], op=OP.mult)
            nc.vector.tensor_tensor(out=ax[:], in0=ax[:], in1=ay[:], op=OP.add)
            sm = work.tile([128, 8, 4], F32, tag="sm")
            nc.scalar.activation(sm[:], ax[:], mybir.ActivationFunctionType.Sqrt,
                                 accum_out=parts[:, 1:2])

            # ---- partition reduce via matmul, then out ----
            ones = aux.tile([128, 1], F32)
            nc.vector.memset(ones[:], 1.0)
            red = psum.tile([8, 1], F32)
            nc.tensor.matmul(red[:], parts[:], ones[:], start=True, stop=True)
            sred = aux.tile([8, 1], F32)
            nc.scalar.copy(out=sred[:], in_=red[:])
            nc.sync.dma_start(partials[:], sred[:])

    nc.compile()
    return nc


def _host_prep(predicted_trajectory, predicted_confidence, target_trajectory,
               detection_mask, drivable_area_mask):
    in_maps = []
    for k in range(NC):
        b0, b1 = k * BC, (k + 1) * BC
        det = detection_mask[b0:b1]
        drv = drivable_area_mask[b0:b1]
        combo = np.zeros((BC, XP, ROWE), dtype=ml_dtypes.bfloat16)
        combo[:, 4:4 + W, 0:H] = det.transpose(0, 2, 1)
        combo[:, 4:4 + W, YP:YP + H] = drv.transpose(0, 2, 1)

        ptx = np.ascontiguousarray(predicted_trajectory[b0:b1, :, 0]).reshape(-1)
        pty = np.ascontiguousarray(predicted_trajectory[b0:b1, :, 1]).reshape(-1)
        cnf = np.ascontiguousarray(predicted_confidence[b0:b1]).reshape(-1)

        pxw = np.clip((ptx[_POINT_W] / 0.5 + 40.0).astype(np.int32), 0, W - 1)
        base = ((_POINT_W // S) % SLICE) * XP
        idx16 = (base + pxw).astype(np.int16)
        pr = predicted_trajectory[b0:b1].reshape(8, 128, 12).transpose(1, 0, 2)
        tg = target_trajectory[b0:b1].reshape(8, 128, 12).transpose(1, 0, 2)
        auxA = np.empty((128, AUXW), np.float32)
        auxA[:, 0:CHUNKS] = ptx[_POINT_P]
        auxA[:, CHUNKS:2 * CHUNKS] = pty[_POINT_P]
        auxA[:, 2 * CHUNKS:3 * CHUNKS] = cnf[_POINT_P]
        auxA[:, 144:240] = pr.reshape(128, 96)
        auxA[:, 240:336] = tg.reshape(128, 96)
        auxA[:, 336:336 + YP] = _IOY
        auxA[:, 336 + YP:] = 0.0

        in_maps.append({
            "combo": combo.reshape(BC * XP, ROWE),
            "auxD": auxA,
            "idxD": idx16,
        })
    return in_maps


def kernel(predicted_trajectory, predicted_confidence, target_trajectory,
           detection_mask, drivable_area_mask):
    global _NC_CACHE, LAST_EXEC_NS
    if _NC_CACHE is None:
        _NC_CACHE = _build_nc()
    nc = _NC_CACHE
    in_maps = _host_prep(predicted_trajectory, predicted_confidence,
                         target_trajectory, detection_mask, drivable_area_mask)
    res = run_bass_kernel_spmd(nc, in_maps, list(range(NC)), trace=TRACE)
    LAST_EXEC_NS = res.exec_time_ns
    tot = np.zeros(8, dtype=np.float64)
    for r in res.results:
        tot += r["partials"].reshape(-1).astype(np.float64)
    traj_loss = tot[0] / (B * S * 2)
    smooth_loss = tot[1] / (B * (S - 2))
    col_loss = tot[2] / (B * S)
    lane_loss = 1.0 - tot[3] / (B * S)
    conf_loss = tot[4] / (B * S)
    total = (W_TRAJ * traj_loss + W_COL * col_loss + W_LANE * lane_loss
             + W_SMOOTH * smooth_loss + W_CONF * conf_loss)
    return np.float32(total)
